# revision 26
# baseline (speedup 1.0000x reference)
"""Trainium2 Bass kernel for nn_Attention_TopM (sparse top-128 attention), v2.

Full-input contract: kernel(x[8,1024,768], Wqkv[2304,768], bqkv[2304]) -> [8,1024,768].
Data-parallel over batch B=8 across 8 NeuronCores (SPMD, no collectives).

v2 redesign vs baseline (1.62ms sim):
  - all big matmuls in fp32r (1 cyc/row vs 4) for proj + scores; bf16 for A@V
  - exp-domain pipeline: scores PSUM -> ACT exp -> et0 = e^s (fp32, SBUF);
    S3 never stored; counts/ladder/mask all on et0
  - count passes: a on ACT (Sign+accum), b/c/d on DVE (tensor_scalar 2x mode)
  - secant threshold iteration with per-row exact mu/sigma init:
    mu via ksum matmul; sigma via PE quadratic form q^T (K^T K) q (bf16)
  - single-sided 8-deep ladder (count window [120,127]): wu-mask+mult on Pool,
    max8 on DVE, one-hot rank pick -> exact rank-128 value; straggler rows
    fall back to nearest-count threshold
  - P = mask*et0 on Pool (bf16); P^T via PE bf16 transposes packed into one
    PSUM bank; single ACT copy; A@V bf16 with ones-column denominator
"""
import sys
import numpy as np

sys.path.insert(0, '/opt/trn_rl_repo')

B, N, C, H, D = 8, 1024, 768, 12, 64
NRT = N // 128          # 8 row tiles per head
NKC = C // 128          # 6 contraction chunks
M3 = 3 * C // 128       # 18 proj row tiles
SCALE = 0.125

Z1 = 1.1735
INV_DENS_C = 1.0 / 205.2
TGT = 123.5
TGTD = 123.5
CLIP_LO, CLIP_HI = 0.5, 2.0
WIN_LO, WIN_HI = 120.0, 127.0

_CACHE = {}


def _build():
    from contextlib import ExitStack
    from concourse import bass, bacc, mybir
    from concourse.tile import TileContext
    from concourse.masks import make_identity

    A = mybir.AluOpType
    AF = mybir.ActivationFunctionType
    F32 = mybir.dt.float32
    F32R = mybir.dt.float32r
    BF16 = mybir.dt.bfloat16
    U8 = mybir.dt.uint8

    nc = bacc.Bacc()
    x_d = nc.declare_dram_parameter("x", [N, C], F32, isOutput=False)
    w_d = nc.declare_dram_parameter("Wqkv", [3 * C, C], F32, isOutput=False)
    b_d = nc.declare_dram_parameter("bqkv", [3 * C], F32, isOutput=False)
    o_d = nc.declare_dram_parameter("out", [N, C], F32, isOutput=True)

    with TileContext(nc) as tc, ExitStack() as ctx:
        const_p = ctx.enter_context(tc.tile_pool(name="const", bufs=1))
        xT_p = ctx.enter_context(tc.tile_pool(name="xT", bufs=1))
        qkvT_p = ctx.enter_context(tc.tile_pool(name="qkvT", bufs=1))
        wrow_p = ctx.enter_context(tc.tile_pool(name="wrow", bufs=2))
        wtb_p = ctx.enter_context(tc.tile_pool(name="wtb", bufs=2))
        bias_p = ctx.enter_context(tc.tile_pool(name="bias", bufs=2))
        et0_p = ctx.enter_context(tc.tile_pool(name="et0", bufs=1))
        jk_p = ctx.enter_context(tc.tile_pool(name="jk", bufs=3))
        wu_p = ctx.enter_context(tc.tile_pool(name="wu", bufs=2))
        mw_p = ctx.enter_context(tc.tile_pool(name="mw", bufs=1))
        P_p = ctx.enter_context(tc.tile_pool(name="P", bufs=2))
        PT_p = ctx.enter_context(tc.tile_pool(name="PT", bufs=2))
        vc_p = ctx.enter_context(tc.tile_pool(name="vc", bufs=1))
        sb_p = ctx.enter_context(tc.tile_pool(name="sbh", bufs=1))
        hs_p = ctx.enter_context(tc.tile_pool(name="hs", bufs=2))
        out_p = ctx.enter_context(tc.tile_pool(name="outsb", bufs=3))

        ps_s = ctx.enter_context(tc.tile_pool(name="ps_s", bufs=2, space="PSUM"))
        ps_t1 = ctx.enter_context(tc.tile_pool(name="ps_t1", bufs=1, space="PSUM"))
        ps_pt = ctx.enter_context(tc.tile_pool(name="ps_pt", bufs=1, space="PSUM"))
        ps_sm = ctx.enter_context(tc.tile_pool(name="ps_sm", bufs=1, space="PSUM"))
        ps_av = ctx.enter_context(tc.tile_pool(name="ps_av", bufs=1, space="PSUM"))

        # ---------------- constants ----------------
        ident = const_p.tile([128, 128], F32, name="ident")
        make_identity(nc, ident)
        identb = const_p.tile([128, 128], BF16, name="identb")
        nc.vector.tensor_copy(identb, ident)
        iota8_i = const_p.tile([128, 8], mybir.dt.int32, name="iota8_i")
        nc.gpsimd.iota(iota8_i, pattern=[[1, 8]], base=0, channel_multiplier=0)
        iota8 = const_p.tile([128, 8], F32, name="iota8")
        nc.vector.tensor_copy(iota8, iota8_i)
        ones128b = const_p.tile([128, 1], BF16, name="ones128b")
        nc.gpsimd.memset(ones128b, 1.0)
        ones64b = ones128b[0:64, :]
        neg128 = const_p.tile([128, 1], F32, name="neg128")
        nc.gpsimd.memset(neg128, -128.0)

        # ---------------- phase A: x -> xT (F32R) ----------------
        xT = [xT_p.tile([128, N], F32R, tag=f"xT{kc}", name=f"xT{kc}")
              for kc in range(NKC)]
        for nt in range(NRT):
            xrow = wrow_p.tile([128, C], F32, tag="xrow", name="xrow")
            nc.sync.dma_start(out=xrow, in_=x_d[nt * 128:(nt + 1) * 128, :])
            for kc in range(NKC):
                tp = ps_pt.tile([128, 128], F32, tag="tpa", name="tpa")
                nc.tensor.transpose(tp, xrow[:, kc * 128:(kc + 1) * 128], ident)
                if kc % 2 == 0:
                    nc.scalar.activation(xT[kc][:, nt * 128:(nt + 1) * 128], tp,
                                         AF.Copy, bias=0.0, scale=1.0)
                else:
                    nc.vector.tensor_copy(xT[kc][:, nt * 128:(nt + 1) * 128], tp)

        # qkvT master tiles: q,k F32R; v BF16
        qkvT = []
        for m in range(M3):
            dt = BF16 if m >= 12 else F32R
            qkvT.append(qkvT_p.tile([128, N], dt, tag=f"qkvT{m}", name=f"qkvT{m}"))

        def emit_proj(m, _noop=None):
            wrow = wrow_p.tile([128, C], F32, tag="wrow", name="wrow")
            nc.sync.dma_start(out=wrow, in_=w_d[m * 128:(m + 1) * 128, :])
            btile = bias_p.tile([128, 1], F32, tag="b", name="btile")
            nc.sync.dma_start(out=btile, in_=b_d[m * 128:(m + 1) * 128])
            is_q = m < NKC
            bscaled = bias_p.tile([128, 1], F32, tag="bs", name="bscaled")
            nc.vector.tensor_scalar_mul(bscaled, btile, SCALE if is_q else 1.0)
            wtb = [wtb_p.tile([128, 128], F32R, tag=f"wtb{kc}", name=f"wtb{kc}")
                   for kc in range(NKC)]
            for kc in range(NKC):
                tp = ps_pt.tile([128, 128], F32, tag="tpa", name="tpb")
                nc.tensor.transpose(tp, wrow[:, kc * 128:(kc + 1) * 128], ident)
                nc.vector.tensor_copy(wtb[kc], tp)
            for nh in range(2):
                pp = ps_s.tile([128, 512], F32, tag="sph", name="pp")
                for kc in range(NKC):
                    nc.tensor.matmul(out=pp, lhsT=wtb[kc],
                                     rhs=xT[kc][:, nh * 512:(nh + 1) * 512],
                                     start=(kc == 0), stop=(kc == NKC - 1))
                nc.scalar.activation(qkvT[m][:, nh * 512:(nh + 1) * 512], pp,
                                     AF.Identity, bias=bscaled,
                                     scale=SCALE if is_q else 1.0)

        from contextlib import contextmanager

        def _dep(it):
            for x in it:
                with tc.high_priority(offset=-300):
                    yield x

        def _dep2(it):
            for x in it:
                with tc.high_priority(offset=-100):
                    yield x

        def emit_head(h):
            hb = h % 2
            qm, off = h // 2, (h % 2) * 64
            qT, kT, vT = qkvT[qm], qkvT[6 + qm], qkvT[12 + qm]

            # ---- bf16 copies for the sigma route ----
            qTb = sb_p.tile([128, N], BF16, tag="qTb", name="qTb")
            nc.gpsimd.tensor_copy(qTb[0:64, :], qT[off:off + 64, :])
            kTb = sb_p.tile([128, N], BF16, tag="kTb", name="kTb")
            nc.gpsimd.tensor_copy(kTb[0:64, :], kT[off:off + 64, :])

            # K and V chunks [128m, 64d] bf16, packed into one PSUM bank
            kv_ps = ps_pt.tile([128, N], BF16, tag="kvps", name="kv_ps")
            kc_ps = kv_ps[:, 0:512]
            vc_ps = kv_ps[:, 512:1024]
            for c in range(NRT):
                nc.tensor.matmul(out=kc_ps[:, (c % 8) * 64:(c % 8) * 64 + 64],
                                 lhsT=kTb[0:64, c * 128:(c + 1) * 128],
                                 rhs=identb[0:64, 0:64], is_transpose=True)
            Kcb = sb_p.tile([128, 512], BF16, tag="Kcb", name="Kcb")
            nc.vector.tensor_copy(Kcb, kc_ps)
            for c in range(NRT):
                nc.tensor.matmul(out=vc_ps[:, c * 64:c * 64 + 64],
                                 lhsT=vT[off:off + 64, c * 128:(c + 1) * 128],
                                 rhs=identb[off:off + 64, off:off + 64],
                                 is_transpose=True)
            Vc = vc_p.tile([128, NRT * 65], BF16, tag=f"Vc{hb}", name="Vc")
            # copy chunks into Vc[:, c*65 : c*65+64]; col 64 of each = ones
            for c in range(NRT):
                nc.vector.tensor_copy(Vc[:, c * 65:c * 65 + 64],
                                      vc_ps[:, c * 64:c * 64 + 64])
            for c in range(NRT):
                nc.gpsimd.memset(Vc[:, c * 65 + 64:c * 65 + 65], 1.0)

            # packed small psum bank: mup 0:8, scol 8:16, ks 16:17, C' 32:96
            smh = ps_sm.tile([128, 128], F32, tag="smh", name="smh")
            ks_ps = smh[0:64, 16:17]
            for c in range(NRT):
                nc.tensor.matmul(out=ks_ps, lhsT=Kcb[:, c * 64:(c + 1) * 64],
                                 rhs=ones128b, start=(c == 0), stop=(c == NRT - 1))
            ksum_b = hs_p.tile([128, 1], BF16, tag="ksr", name="ksum_b")
            nc.vector.tensor_copy(ksum_b[0:64, :], ks_ps)

            # C' = K^T K [64,64] bf16
            cp_ps = smh[0:64, 32:96]
            for c in range(NRT):
                nc.tensor.matmul(out=cp_ps, lhsT=Kcb[:, c * 64:(c + 1) * 64],
                                 rhs=Kcb[:, c * 64:(c + 1) * 64],
                                 start=(c == 0), stop=(c == NRT - 1))
            Cb = hs_p.tile([128, 64], BF16, tag="Cb", name="Cb")
            nc.vector.tensor_copy(Cb[0:64, :], cp_ps)

            # T1 = C @ qTb [64, N] ; U = qTb * T1
            Ub = sb_p.tile([128, N], BF16, tag="Ub", name="Ub")
            for nh in range(2):
                t1_ps = ps_t1.tile([64, 512], F32, tag="t1", name="t1_ps")
                nc.tensor.matmul(out=t1_ps,
                                 lhsT=Cb[0:64, :],
                                 rhs=qTb[0:64, nh * 512:(nh + 1) * 512],
                                 start=True, stop=True)
                nc.vector.tensor_mul(Ub[0:64, nh * 512:(nh + 1) * 512],
                                     qTb[0:64, nh * 512:(nh + 1) * 512], t1_ps)

            # mu, ex2 columns per row tile
            mup = smh[:, 0:NRT]
            for rt in range(NRT):
                nc.tensor.matmul(out=mup[:, rt:rt + 1],
                                 lhsT=qTb[0:64, rt * 128:(rt + 1) * 128],
                                 rhs=ksum_b[0:64, :], start=True, stop=True)
            scol = smh[:, 8:8 + NRT]
            for rt in range(NRT):
                nc.tensor.matmul(out=scol[:, rt:rt + 1],
                                 lhsT=Ub[0:64, rt * 128:(rt + 1) * 128],
                                 rhs=ones64b, start=True, stop=True)

            # ---- smalls: mu, sigma, t_a, invd ----
            mu = hs_p.tile([128, NRT], F32, tag="mu", name="mu")
            nc.vector.tensor_scalar_mul(mu, mup, 1.0 / N)
            ex2 = hs_p.tile([128, NRT], F32, tag="ex2", name="ex2")
            nc.vector.tensor_scalar_mul(ex2, scol, 1.0 / N)
            mu2 = hs_p.tile([128, NRT], F32, tag="mu2", name="mu2")
            nc.vector.tensor_mul(mu2, mu, mu)
            var = hs_p.tile([128, NRT], F32, tag="var", name="var")
            nc.vector.tensor_sub(var, ex2, mu2)
            yi = hs_p.tile([128, NRT], mybir.dt.int32, tag="yi", name="yi")
            nc.vector.tensor_scalar(out=yi, in0=var.bitcast(mybir.dt.int32),
                                    scalar1=1, scalar2=None,
                                    op0=A.logical_shift_right)
            nc.vector.tensor_scalar(out=yi, in0=yi, scalar1=-1,
                                    scalar2=0x5f3759df, op0=A.mult, op1=A.add)
            yt = hs_p.tile([128, NRT], F32, tag="yt", name="yt")
            nc.vector.tensor_mul(yt, yi.bitcast(F32), yi.bitcast(F32))
            nc.vector.tensor_mul(yt, yt, var)
            nc.vector.tensor_scalar(out=yt, in0=yt, scalar1=-0.5, scalar2=1.5,
                                    op0=A.mult, op1=A.add)
            sig = hs_p.tile([128, NRT], F32, tag="sig", name="sig")
            nc.vector.tensor_mul(sig, yi.bitcast(F32), yt)
            nc.vector.tensor_mul(sig, sig, var)
            invd = hs_p.tile([128, NRT], F32, tag="invd", name="invd")
            nc.vector.tensor_scalar_mul(invd, sig, INV_DENS_C)
            rinvd = hs_p.tile([128, NRT], F32, tag="rinvd", name="rinvd")
            nc.vector.reciprocal(rinvd, invd)
            t_a = hs_p.tile([128, NRT], F32, tag="t_a", name="t_a")
            nc.vector.tensor_scalar(out=t_a, in0=sig, scalar1=Z1, scalar2=None,
                                    op0=A.mult)
            nc.vector.tensor_add(t_a, t_a, mu)

            # ---- scores + exp per row tile ----
            et0 = []
            for rt in range(NRT):
                et = et0_p.tile([128, N], F32, tag=f"et{hb}_{rt}", name=f"et{rt}")
                for nh in range(2):
                    sp = ps_s.tile([128, 512], F32, tag="sph", name="sp")
                    nc.tensor.matmul(out=sp,
                                     lhsT=qT[off:off + 64, rt * 128:(rt + 1) * 128],
                                     rhs=kT[off:off + 64, nh * 512:(nh + 1) * 512],
                                     start=True, stop=True)
                    nc.scalar.activation(et[:, nh * 512:(nh + 1) * 512], sp,
                                         AF.Exp, bias=0.0, scale=1.0)
                et0.append(et)

            def exp_small(tin, nm):
                tout = hs_p.tile([128, NRT], F32, tag=nm, name=nm)
                nc.scalar.activation(tout, tin, AF.Exp, bias=0.0, scale=1.0)
                return tout

            # ---- count a: ACT sign ----
            tau_a = exp_small(t_a, "tau_a")
            ntau_a = hs_p.tile([128, NRT], F32, tag="ntau_a", name="ntau_a")
            nc.vector.tensor_scalar_mul(ntau_a, tau_a, -1.0)
            sa = hs_p.tile([128, NRT], F32, tag="sa", name="sa")
            for rt in range(NRT):
                jk = jk_p.tile([128, N], BF16, tag="jk", name="jk")
                nc.scalar.activation(jk, et0[rt], AF.Sign,
                                     bias=ntau_a[:, rt:rt + 1], scale=1.0,
                                     accum_out=sa[:, rt:rt + 1])
            c_a = hs_p.tile([128, NRT], F32, tag="c_a", name="c_a")
            nc.vector.tensor_scalar(out=c_a, in0=sa, scalar1=0.5,
                                    scalar2=float(N) / 2.0, op0=A.mult, op1=A.add)

            def secant(tprev, cprev, tgt, stepv, nm):
                stp = hs_p.tile([128, NRT], F32, tag=nm + "s", name=nm + "s")
                nc.vector.scalar_tensor_tensor(out=stp, in0=cprev, scalar=-tgt,
                                               in1=stepv, op0=A.add, op1=A.mult)
                tn = hs_p.tile([128, NRT], F32, tag=nm, name=nm)
                nc.vector.tensor_add(tn, tprev, stp)
                return tn

            def count_dve(taus, nm):
                cc = hs_p.tile([128, NRT], F32, tag=nm, name=nm)
                for rt in range(NRT):
                    jk = jk_p.tile([128, N], BF16, tag="jk", name="jk2")
                    nc.vector.tensor_scalar(out=jk, in0=et0[rt],
                                            scalar1=taus[:, rt:rt + 1],
                                            scalar2=None, op0=A.is_ge, op1=A.add,
                                            accum_out=cc[:, rt:rt + 1])
                return cc

            def obs_dens(t2, t1v, c2, c1v, nm):
                # |dt|/max(|dc|,0.5), clamped to [CLIP_LO, CLIP_HI]*invd
                dt = hs_p.tile([128, NRT], F32, tag=nm + "dt", name=nm + "dt")
                nc.vector.tensor_sub(dt, t2, t1v)
                nc.scalar.activation(dt, dt, AF.Abs, bias=0.0, scale=1.0)
                dc = hs_p.tile([128, NRT], F32, tag=nm + "dc", name=nm + "dc")
                nc.vector.tensor_sub(dc, c2, c1v)
                nc.scalar.activation(dc, dc, AF.Abs, bias=0.0, scale=1.0)
                nc.vector.tensor_scalar_max(dc, dc, 0.5)
                radc = hs_p.tile([128, NRT], F32, tag=nm + "r", name=nm + "r")
                nc.vector.reciprocal(radc, dc)
                iobs = hs_p.tile([128, NRT], F32, tag=nm + "i", name=nm + "i")
                nc.vector.tensor_mul(iobs, dt, radc)
                nc.vector.tensor_mul(iobs, iobs, rinvd)
                nc.vector.tensor_scalar(out=iobs, in0=iobs, scalar1=CLIP_LO,
                                        scalar2=CLIP_HI, op0=A.max, op1=A.min)
                nc.vector.tensor_mul(iobs, iobs, invd)
                return iobs

            t_b = secant(t_a, c_a, TGT, invd, "t_b")
            tau_b = exp_small(t_b, "tau_b")
            c_b = count_dve(tau_b, "c_b")

            i1 = obs_dens(t_b, t_a, c_b, c_a, "i1")
            t_c = secant(t_b, c_b, TGT, i1, "t_c")
            tau_c = exp_small(t_c, "tau_c")
            c_c = count_dve(tau_c, "c_c")

            i2 = obs_dens(t_c, t_b, c_c, c_b, "i2")
            t_d = secant(t_c, c_c, TGTD, i2, "t_d")
            tau_d = exp_small(t_d, "tau_d")
            c_d = count_dve(tau_d, "c_d")

            # ---- keeper: window (later passes override) + nearest fallback ----
            tk_w = hs_p.tile([128, NRT], F32, tag="tk_w", name="tk_w")
            nc.vector.tensor_copy(tk_w, tau_d)
            ck_w = hs_p.tile([128, NRT], F32, tag="ck_w", name="ck_w")
            nc.vector.memset(ck_w, -1.0)

            def keep_update(tauv, cv, idx):
                o1 = hs_p.tile([128, NRT], F32, tag=f"o1{idx}", name=f"o1{idx}")
                nc.vector.tensor_scalar(out=o1, in0=cv, scalar1=WIN_LO,
                                        scalar2=None, op0=A.is_ge)
                ok = hs_p.tile([128, NRT], U8, tag=f"ok{idx}", name=f"ok{idx}")
                nc.vector.scalar_tensor_tensor(out=ok, in0=cv, scalar=WIN_HI,
                                               in1=o1, op0=A.is_le, op1=A.mult)
                nc.vector.copy_predicated(tk_w, ok, tauv)
                nc.vector.copy_predicated(ck_w, ok, cv)

            keep_update(tau_b, c_b, 0)
            keep_update(tau_c, c_c, 1)
            keep_update(tau_d, c_d, 2)

            sg = hs_p.tile([128, NRT], U8, tag="sg", name="sg")
            nc.vector.tensor_scalar(out=sg, in0=ck_w, scalar1=0.0, scalar2=None,
                                    op0=A.is_lt)

            kst = hs_p.tile([128, NRT], F32, tag="kst", name="kst")
            nc.vector.tensor_scalar(out=kst, in0=ck_w, scalar1=-1.0,
                                    scalar2=float(WIN_HI), op0=A.mult, op1=A.add)
            nc.vector.tensor_scalar(out=kst, in0=kst, scalar1=0.0,
                                    scalar2=7.0, op0=A.max, op1=A.min)

            # ---- ladder: wu = [et0 < tk_w]*et0 (Pool), max8 (DVE) ----
            m8h = hs_p.tile([128, NRT * 8], F32, tag="m8h", name="m8h")
            for rt in range(NRT):
                mw = mw_p.tile([128, N], BF16, tag="mw", name="mw")
                nc.gpsimd.tensor_scalar(out=mw, in0=et0[rt],
                                        scalar1=tk_w[:, rt:rt + 1],
                                        scalar2=None, op0=A.is_lt)
                wu = wu_p.tile([128, N], F32, tag="wu", name="wu")
                nc.gpsimd.tensor_tensor(out=wu, in0=et0[rt], in1=mw, op=A.mult)
                nc.vector.max(out=m8h[:, rt * 8:rt * 8 + 8], in_=wu)

            # one-hot rank pick -> ustar
            ustar = hs_p.tile([128, NRT], F32, tag="ustar", name="ustar")
            for rt in range(NRT):
                oh = hs_p.tile([128, 8], F32, tag="oh", name="oh")
                nc.vector.tensor_scalar(out=oh, in0=iota8,
                                        scalar1=kst[:, rt:rt + 1], scalar2=None,
                                        op0=A.is_equal)
                pick = hs_p.tile([128, 8], F32, tag="pick", name="pick")
                nc.vector.scalar_tensor_tensor(out=pick,
                                               in0=m8h[:, rt * 8:rt * 8 + 8],
                                               scalar=0.0, in1=oh, op0=A.add,
                                               op1=A.mult,
                                               accum_out=ustar[:, rt:rt + 1])
            nc.vector.copy_predicated(ustar, sg, tk_w)

            # ---- P = [et0 >= ustar]*et0 (Pool, bf16) ----
            for rt in _dep(range(NRT)):
                mf = mw_p.tile([128, N], BF16, tag="mf", name="mf")
                nc.gpsimd.tensor_scalar(out=mf, in0=et0[rt],
                                        scalar1=ustar[:, rt:rt + 1],
                                        scalar2=None, op0=A.is_ge)
                Pt = P_p.tile([128, N], BF16, tag="P", name="Pt")
                nc.gpsimd.tensor_tensor(out=Pt, in0=mf, in1=et0[rt], op=A.mult)

                # P^T via PE bf16 transposes -> one PSUM bank -> ACT copy
                ptp = ps_pt.tile([128, N], BF16, tag="ptp", name="ptp")
                for c in range(NRT):
                    nc.tensor.transpose(ptp[:, c * 128:(c + 1) * 128],
                                        Pt[:, c * 128:(c + 1) * 128], identb)
                PTs = PT_p.tile([128, N], BF16, tag="PTs", name="PTs")
                nc.scalar.activation(PTs, ptp, AF.Copy, bias=0.0, scale=1.0)

                avp = ps_av.tile([128, 65], F32, tag="av", name="avp")
                for c in range(NRT):
                    nc.tensor.matmul(out=avp,
                                     lhsT=PTs[:, c * 128:(c + 1) * 128],
                                     rhs=Vc[:, c * 65:(c + 1) * 65],
                                     start=(c == 0), stop=(c == NRT - 1))

                dsb = hs_p.tile([128, 1], F32, tag="dsb", name="dsb")
                nc.vector.tensor_copy(dsb, avp[:, 64:65])
                rden = hs_p.tile([128, 1], F32, tag="rden", name="rden")
                nc.vector.reciprocal(rden, dsb)
                otile = out_p.tile([128, 64], F32, tag="ot", name="otile")
                nc.vector.tensor_scalar(out=otile, in0=avp[:, 0:64],
                                        scalar1=rden, scalar2=None, op0=A.mult)
                nc.sync.dma_start(
                    out=o_d[rt * 128:(rt + 1) * 128, h * 64:(h + 1) * 64],
                    in_=otile)

        # interleave proj pairs with heads for cross-phase overlap
        for p in range(6):
            with tc.high_priority(offset=-300):
                emit_proj(p)
                emit_proj(6 + p)
                emit_proj(12 + p)
            emit_head(2 * p)
            emit_head(2 * p + 1)

    nc.finalize()
    return nc


def _get_nc():
    if 'nc' not in _CACHE:
        _CACHE['nc'] = _build()
    return _CACHE['nc']


def kernel(x, Wqkv, bqkv):
    from concourse.bass_utils import run_bass_kernel_spmd
    nc = _get_nc()
    x = np.ascontiguousarray(np.asarray(x, np.float32))
    W = np.ascontiguousarray(np.asarray(Wqkv, np.float32))
    bq = np.ascontiguousarray(np.asarray(bqkv, np.float32))
    in_maps = [{"x": x[i], "Wqkv": W, "bqkv": bq} for i in range(B)]
    res = run_bass_kernel_spmd(nc, in_maps, list(range(B)))
    out = np.stack([np.asarray(res.results[i]["out"]) for i in range(B)])
    return out.astype(np.float32)
